# revision 63
# baseline (speedup 1.0000x reference)
"""Causal multi-head attention for Trainium2, sharded over 8 NeuronCores.

Problem: Q,K,V [2, 16, 2048, 128] fp32 -> O [2, 16, 2048, 128] fp32
  scores = (Q @ K^T) / sqrt(128), causal mask, softmax, @ V.

Sharding: the 32 (batch, head) slices are data-parallel; each of the 8
cores computes 4 heads independently (no collectives). Q and K are
pre-transposed on the host to [head, d, s] so the device needs no
transposes at all (the PE contraction dim d lands on partitions).

Per-head dataflow on one core (S=2048, D=128, bf16 matmuls, fp32 psum):
  load Qt,Kt [d, s] fp32 -> DVE cast bf16 (in halves); V loads [s, d] and
  DVE-casts to bf16 with a ones column appended (softmax denominator rides
  along mm2). mm1 computes scores^T [k, q] only over the causal region,
  packed into a flat 17408-col buffer (block i occupies cols
  off(i)..off(i)+2048-128*i), 512-col bank-aligned sub-matmuls; each
  diagonal block's psum is pre-seeded with -1e30 on the strict lower
  triangle so exp emits exact zeros there. ACT exps 1536-col chunks (12
  per head, scale folded, fp32 in / bf16 out, no max-subtraction: scores
  are O(+-8)). mm2 per 128-row output block b accumulates pt-stationary
  matmuls over [V | 1]; reciprocal+normalize batched 3 blocks per psum
  bank; stores ride the GPSIMD SWDGE queue.

Queues: Sync = input loads only, Scalar = exp only, GPSIMD = stores,
DVE = casts + normalize, PE = mm1 + seeds + mm2 (+ warmup during loads).
A global mm2 cursor paces mm2 a few chunks behind exp; loads prefetch two
heads ahead; PE warmup and the exp-table load run during the first loads.
"""

import math
from contextlib import ExitStack

import numpy as np

N_CORES = 8
B, H, S, D = 2, 16, 2048, 128
HEADS_PER_CORE = (B * H) // N_CORES  # 4
SB = S // 128  # 16 k-blocks per head
SCALE = 1.0 / math.sqrt(128.0)
CHUNK = 1536
FLAT = sum(S - 128 * i for i in range(SB))  # 17408
NCH = (FLAT + CHUNK - 1) // CHUNK  # 12 exp chunks per head
N_WARM = 60  # PE warmup matmuls (post-program-load, pre-first-mm1)
MM2_BUDGET = 13  # mm2 matmuls emitted per chunk step

_CACHE = {}


def _off(i):
    # flat column offset of k-block i's causal q-range (width S - 128*i)
    return 2048 * i - 64 * i * (i - 1)


def _build():
    import concourse.bass as bass  # noqa: F401
    import concourse.tile as tile
    from concourse import bacc, mybir

    f32 = mybir.dt.float32
    bf16 = mybir.dt.bfloat16

    nc = bacc.Bacc("TRN2", num_devices=N_CORES)
    # C: host consts: [0]=identity, [1]=-1e30 strict lower tri, [2]=0/1 keep
    Cd = nc.declare_dram_parameter("C", [3, 128, 128], f32, isOutput=False)
    Qd = nc.declare_dram_parameter("Q", [HEADS_PER_CORE, D, S], f32, isOutput=False)
    Kd = nc.declare_dram_parameter("K", [HEADS_PER_CORE, D, S], f32, isOutput=False)
    # V host-relaid as [head, p, o, d] with s = o*128 + p so each SBUF
    # partition's load is one contiguous 8KB run.
    Vd = nc.declare_dram_parameter(
        "V", [HEADS_PER_CORE, 128, SB, D], f32, isOutput=False
    )
    # O stored partition-major [head, p, o, d] (s = o*128 + p) so each store
    # is one contiguous 8KB run per partition; host un-permutes after gather.
    Od = nc.declare_dram_parameter(
        "O", [HEADS_PER_CORE, 128, SB, D], f32, isOutput=True
    )

    # mm2 normalize/store groups of consecutive output blocks (3 fit a bank)
    GROUPS = [[0, 1, 2], [3, 4, 5], [6, 7, 8], [9, 10, 11], [12, 13, 14], [15]]

    with tile.TileContext(nc) as tc, ExitStack() as ctx:
        sb_pool = ctx.enter_context(tc.tile_pool(name="sb", bufs=2))
        o_pool = ctx.enter_context(tc.tile_pool(name="op", bufs=2))
        ps_pool = ctx.enter_context(tc.tile_pool(name="psp", bufs=2, space="PSUM"))
        po_pool = ctx.enter_context(tc.tile_pool(name="pop", bufs=2, space="PSUM"))
        const = in_pool = qk_pool = vp_pool = pt_pool = s_pool = sb_pool

        cf = const.tile([128, 3, 128], f32)
        eye = const.tile([128, 128], bf16)
        negtri = const.tile([128, 128], bf16)
        tri01 = const.tile([128, 128], bf16)
        warm_w = const.tile([128, 128], bf16)

        state = {}
        ps_tiles = {}

        def emit_loads(h):
            qtf = in_pool.tile([128, S], f32, tag="qtf")
            nc.sync.dma_start(qtf[:], Qd.ap()[h])
            ktf = in_pool.tile([128, S], f32, tag="ktf")
            nc.sync.dma_start(ktf[:], Kd.ap()[h])
            vn = in_pool.tile([128, SB, D], f32, tag="vn")
            nc.sync.dma_start(vn[:], Vd.ap()[h])
            state[h] = {"qtf": qtf, "ktf": ktf, "vn": vn}

        def emit_cast_q(h, half):
            st = state[h]
            if half == 0:
                st["qtb"] = qk_pool.tile([128, S], bf16, tag="qtb", name="qtb")
            sl = slice(half * (S // 2), (half + 1) * (S // 2))
            nc.vector.tensor_copy(st["qtb"][:, sl], st["qtf"][:, sl])

        def emit_cast_q_piece(h, c0, c1):
            st = state[h]
            if c0 == 0:
                st["qtb"] = qk_pool.tile([128, S], bf16, tag="qtb", name="qtb")
            nc.vector.tensor_copy(st["qtb"][:, c0:c1], st["qtf"][:, c0:c1])

        def emit_cast_k(h, half):
            st = state[h]
            if half == 0:
                st["ktb"] = qk_pool.tile([128, S], bf16, tag="ktb", name="ktb")
            sl = slice(half * (S // 2), (half + 1) * (S // 2))
            nc.vector.tensor_copy(st["ktb"][:, sl], st["ktf"][:, sl])

        def emit_cast_k_piece(h, c0, c1):
            st = state[h]
            if c0 == 0:
                st["ktb"] = qk_pool.tile([128, S], bf16, tag="ktb", name="ktb")
            nc.vector.tensor_copy(st["ktb"][:, c0:c1], st["ktf"][:, c0:c1])

        def emit_cast_v(h):
            st = state[h]
            vp = vp_pool.tile([128, SB, D + 4], bf16, tag="vp")
            nc.vector.tensor_copy(vp[:, :, 0:D], st["vn"][:])
            if h < 2:
                # the ones column survives slot reuse (casts only write 0:D)
                nc.vector.memset(vp[:, :, D : D + 1], 1.0)
            st["vp"] = vp

        # ---- mm2 job stream: one op per (block, contraction i) matmul, with
        # group-finalize ops (reciprocal + normalize + store) interleaved.
        # ready = global chunk step at which the needed pt slice is exp'd,
        # floored so a chain doesn't start long before its diagonal (keeps
        # the po3 psum slot hold short), plus a 2-step pipeline lag.
        def build_mm2_ops(h):
            ops = []
            for grp in GROUPS:
                for j, b in enumerate(grp):
                    rc_diag = _off(b) // CHUNK
                    lag = 2 if h + 1 < HEADS_PER_CORE else 0
                    for i in range(b + 1):
                        pos_rc = (_off(i) + 128 * (b - i)) // CHUNK
                        rdy = NCH * h + max(pos_rc, rc_diag - 3) + lag
                        ops.append((rdy, "mm", h, grp[0], len(grp), j, b, i))
                ops.append((ops[-1][0], "fin", h, grp[0], len(grp), 0, 0, 0))
            return ops

        mm2_ops = []
        for h in range(HEADS_PER_CORE):
            mm2_ops.extend(build_mm2_ops(h))
        mm2_cursor = [0]

        def emit_mm2(gstep, budget):
            cur = mm2_cursor[0]
            while cur < len(mm2_ops):
                rdy, kind, h, b0, glen, j, b, i = mm2_ops[cur]
                if rdy > gstep or (budget <= 0 and kind == "mm"):
                    break
                st = state[h]
                if kind == "mm":
                    if j == 0 and i == 0:
                        st["po3"] = po_pool.tile(
                            [128, 3, D + 4], f32, tag="po3", name="po3"
                        )
                    pos = _off(i) + 128 * (b - i)
                    nc.tensor.matmul(
                        st["po3"][:, j, 0 : D + 1],
                        lhsT=st["pt"][:, pos : pos + 128],
                        rhs=st["vp"][:, i, 0 : D + 1],
                        start=(i == 0),
                        stop=(i == b),
                        skip_group_check=True,
                    )
                    budget -= 1
                else:
                    po3 = st["po3"]
                    rec = s_pool.tile([128, 3], f32, tag="rec")
                    nc.vector.reciprocal(rec[:, 0:glen], po3[:, 0:glen, D])
                    if b0 == 0:
                        st["ob"] = o_pool.tile(
                            [128, SB, D], f32, tag="ob", name="ob"
                        )
                    nc.vector.tensor_tensor(
                        st["ob"][:, b0 : b0 + glen, :],
                        po3[:, 0:glen, 0:D],
                        rec[:, 0:glen, None].to_broadcast((128, glen, D)),
                        mybir.AluOpType.mult,
                    )
                    if b0 + glen == SB:
                        # one store per head: keeps the store queue shallow so
                        # the final store (which gates kernel end) fires fast
                        nc.gpsimd.dma_start(Od.ap()[h], st["ob"][:])
                cur += 1
            mm2_cursor[0] = cur

        def emit_mm1(h, c):
            st = state[h]
            if c == 0:
                st["pt"] = pt_pool.tile([128, FLAT], bf16, tag="pt", name="pt")
            qtb, ktb = st["qtb"], st["ktb"]
            s0 = CHUNK * c
            s1 = min(CHUNK * (c + 1), FLAT)
            ps = ps_pool.tile([128, CHUNK], f32, tag="ps")
            ps_tiles[NCH * h + c] = ps
            first_head_start = False
            # mm1: bank-aligned sub-matmuls over causal blocks in this chunk.
            # A block run starting at its diagonal gets a -1e30 seed first;
            # the first sub-matmul accumulates onto it (start=False). The
            # very first chunk skips its seed (no const dependency at start);
            # its diagonal is masked post-exp on the DVE instead.
            for i in range(SB):
                a = max(_off(i), s0)
                bnd = min(_off(i) + (S - 128 * i), s1)
                if a >= bnd:
                    continue
                seeded = a == _off(i) and not (h == 0 and c == 0)
                if seeded:
                    nc.tensor.matmul(
                        ps[:, a - s0 : a - s0 + 128],
                        lhsT=eye[:],
                        rhs=negtri[:],
                        start=True,
                        stop=False,
                        skip_group_check=True,
                    )
                f = a
                while f < bnd:
                    nxt = min(bnd, (f // 512 + 1) * 512)
                    q0 = 128 * i + (f - _off(i))
                    nc.tensor.matmul(
                        ps[:, f - s0 : nxt - s0],
                        lhsT=ktb[:, 128 * i : 128 * i + 128],
                        rhs=qtb[:, q0 : q0 + (nxt - f)],
                        start=not (seeded and f == a),
                        stop=True,
                        skip_group_check=True,
                    )
                    if first_head_start:
                        # stream exp piecewise so ACT starts ASAP at kernel start
                        nc.scalar.activation(
                            st["pt"][:, f:nxt],
                            ps[:, f - s0 : nxt - s0],
                            mybir.ActivationFunctionType.Exp,
                            scale=SCALE,
                        )
                    f = nxt

        def emit_exp(h, c):
            st = state[h]
            s0 = CHUNK * c
            s1 = min(CHUNK * (c + 1), FLAT)
            ps = ps_tiles.pop(NCH * h + c)
            nc.scalar.activation(
                st["pt"][:, s0:s1],
                ps[:, 0 : s1 - s0],
                mybir.ActivationFunctionType.Exp,
                scale=SCALE,
            )

        def emit_step(h, c):
            """Step g runs mm1 for chunk g+1 (one step ahead, so the mm2
            batch never sits between an exp and the mm1 it waits on), then
            exp for chunk g, then a paced mm2 batch."""
            gstep = NCH * h + c
            if c == 0 and h + 2 < HEADS_PER_CORE:
                emit_loads(h + 2)
            if h + 1 < HEADS_PER_CORE:
                if c == 4:
                    emit_cast_v(h + 1)
                elif c == 6:
                    emit_cast_q(h + 1, 0)
                elif c == 7:
                    emit_cast_q(h + 1, 1)
                elif c == 8:
                    emit_cast_k(h + 1, 0)
                elif c == 9:
                    emit_cast_k(h + 1, 1)

            if gstep > 0:
                emit_mm1(h, c)
                emit_exp(h, c)
            emit_mm2(gstep, MM2_BUDGET)

        # prologue. Ring order: Q0 first (longest pole: load -> cast -> mm1
        # -> exp), then K0's head block, consts, rest of K0, V0, head-1.
        # The PE warmup uses a memset tile so it only depends on GPSIMD.
        st0 = state.setdefault(0, {})
        qtf0 = in_pool.tile([128, S], f32, tag="qtf")
        ktf0 = in_pool.tile([128, S], f32, tag="ktf")
        nc.sync.dma_start(qtf0[:], Qd.ap()[0])
        nc.sync.dma_start(ktf0[:, 0:128], Kd.ap()[0][:, 0:128])
        nc.sync.dma_start(cf[:], Cd.ap().rearrange("c p d -> p c d"))
        nc.sync.dma_start(ktf0[:, 128:S], Kd.ap()[0][:, 128:S])
        vn0 = in_pool.tile([128, SB, D], f32, tag="vn")
        nc.sync.dma_start(vn0[:], Vd.ap()[0])
        st0.update({"qtf": qtf0, "ktf": ktf0, "vn": vn0})
        emit_loads(1)

        nc.gpsimd.memset(warm_w[:], 0.5)
        # load the exp table on ACT during the input loads (tiny dummy exp)
        warm_act = s_pool.tile([128, 1], f32, tag="wa")
        nc.scalar.activation(
            warm_act[:], warm_w[:, 0:1], mybir.ActivationFunctionType.Exp,
            scale=SCALE,
        )
        # PE warmup: start the p-state ramp before the first real matmul
        wps = ps_pool.tile([128, CHUNK], f32, tag="ps")
        for _ in range(N_WARM):
            nc.tensor.matmul(
                wps[:, 0:128], lhsT=warm_w[:], rhs=warm_w[:], start=True,
                stop=True, skip_group_check=True,
            )

        emit_cast_q_piece(0, 0, CHUNK)
        emit_cast_k_piece(0, 0, 128)
        nc.vector.tensor_copy(eye[:], cf[:, 0, :])
        nc.vector.tensor_copy(negtri[:], cf[:, 1, :])
        nc.vector.tensor_copy(tri01[:], cf[:, 2, :])
        emit_cast_q_piece(0, CHUNK, S)
        emit_cast_k_piece(0, 128, 1024)
        emit_cast_k_piece(0, 1024, S)
        emit_cast_v(0)
        emit_mm1(0, 0)
        emit_exp(0, 0)
        # chunk (0,0) carried no seed: mask its diagonal block post-exp
        nc.vector.tensor_tensor(
            st0["pt"][:, 0:128], st0["pt"][:, 0:128], tri01[:],
            mybir.AluOpType.mult,
        )

        for h in range(HEADS_PER_CORE):
            for c in range(NCH):
                emit_step(h, c)
        # drain the mm2 tail
        emit_mm2(10**9, 10**9)

    nc.compile()
    return nc


def _get_nc():
    if "nc" not in _CACHE:
        _CACHE["nc"] = _build()
    return _CACHE["nc"]


def _consts():
    eye = np.eye(128, dtype=np.float32)
    lower = np.arange(128)[:, None] > np.arange(128)[None, :]
    negtri = np.where(lower, -1e30, 0.0).astype(np.float32)
    tri01 = np.where(lower, 0.0, 1.0).astype(np.float32)
    return np.stack([eye, negtri, tri01])


def _in_maps(Q, K, V):
    """Host-side shard + layout prep: Q,K -> [head, d, s], V -> [head, s, d]."""
    Qf = np.asarray(Q, dtype=np.float32).reshape(B * H, S, D)
    Kf = np.asarray(K, dtype=np.float32).reshape(B * H, S, D)
    Vf = np.ascontiguousarray(
        np.asarray(V, dtype=np.float32)
        .reshape(B * H, SB, 128, D)
        .transpose(0, 2, 1, 3)
    )
    Qt = np.ascontiguousarray(Qf.transpose(0, 2, 1))
    Kt = np.ascontiguousarray(Kf.transpose(0, 2, 1))
    C = _consts()
    maps = []
    for c in range(N_CORES):
        sl = slice(c * HEADS_PER_CORE, (c + 1) * HEADS_PER_CORE)
        maps.append({"C": C, "Q": Qt[sl], "K": Kt[sl], "V": Vf[sl]})
    return maps


def _gather(res):
    out = np.concatenate(
        [res.results[c]["O"] for c in range(N_CORES)], axis=0
    )
    # [bh, p, o, d] -> [bh, s, d] with s = o*128 + p
    out = out.reshape(B * H, 128, SB, D).transpose(0, 2, 1, 3)
    return np.ascontiguousarray(out).reshape(B, H, S, D).astype(np.float32)


def kernel(Q: np.ndarray, K: np.ndarray, V: np.ndarray) -> np.ndarray:
    from concourse.bass_utils import run_bass_kernel_spmd

    nc = _get_nc()
    res = run_bass_kernel_spmd(nc, _in_maps(Q, K, V), core_ids=list(range(N_CORES)))
    return _gather(res)


# revision 64
# speedup vs baseline: 1.1712x; 1.1712x over previous
"""Causal multi-head attention for Trainium2, sharded over 8 NeuronCores.

Problem: Q,K,V [2, 16, 2048, 128] fp32 -> O [2, 16, 2048, 128] fp32
  scores = (Q @ K^T) / sqrt(128), causal mask, softmax, @ V.

Sharding: the 32 (batch, head) slices are data-parallel; each of the 8
cores computes 4 heads independently (no collectives). Q and K are
pre-transposed on the host to [head, d, s] so the device needs no
transposes at all (the PE contraction dim d lands on partitions).

Per-head dataflow on one core (S=2048, D=128, bf16 matmuls, fp32 psum):
  load Qt,Kt [d, s] fp32 -> DVE cast bf16 (in halves); V loads [s, d] and
  DVE-casts to bf16 with a ones column appended (softmax denominator rides
  along mm2). mm1 computes scores^T [k, q] only over the causal region,
  packed into a flat 17408-col buffer (block i occupies cols
  off(i)..off(i)+2048-128*i), 512-col bank-aligned sub-matmuls; each
  diagonal block's psum is pre-seeded with -1e30 on the strict lower
  triangle so exp emits exact zeros there. ACT exps 1536-col chunks (12
  per head, scale folded, fp32 in / bf16 out, no max-subtraction: scores
  are O(+-8)). mm2 per 128-row output block b accumulates pt-stationary
  matmuls over [V | 1]; reciprocal+normalize batched 3 blocks per psum
  bank; stores ride the GPSIMD SWDGE queue.

Queues: Sync = input loads only, Scalar = exp only, GPSIMD = stores,
DVE = casts + normalize, PE = mm1 + seeds + mm2 (+ warmup during loads).
A global mm2 cursor paces mm2 a few chunks behind exp; loads prefetch two
heads ahead; PE warmup and the exp-table load run during the first loads.
"""

import math
from contextlib import ExitStack

import numpy as np

N_CORES = 8
B, H, S, D = 2, 16, 2048, 128
HEADS_PER_CORE = (B * H) // N_CORES  # 4
SB = S // 128  # 16 k-blocks per head
SCALE = 1.0 / math.sqrt(128.0)
CHUNK = 1536
FLAT = sum(S - 128 * i for i in range(SB))  # 17408
NCH = (FLAT + CHUNK - 1) // CHUNK  # 12 exp chunks per head
N_WARM = 60  # PE warmup matmuls (post-program-load, pre-first-mm1)
MM2_BUDGET = 13  # mm2 matmuls emitted per chunk step

_CACHE = {}


def _off(i):
    # flat column offset of k-block i's causal q-range (width S - 128*i)
    return 2048 * i - 64 * i * (i - 1)


def _build():
    import concourse.bass as bass  # noqa: F401
    import concourse.tile as tile
    from concourse import bacc, mybir

    f32 = mybir.dt.float32
    bf16 = mybir.dt.bfloat16

    nc = bacc.Bacc("TRN2", num_devices=N_CORES)
    # C: host consts: [0]=identity, [1]=-1e30 strict lower tri, [2]=0/1 keep
    Cd = nc.declare_dram_parameter("C", [3, 128, 128], f32, isOutput=False)
    Qd = nc.declare_dram_parameter("Q", [HEADS_PER_CORE, D, S], f32, isOutput=False)
    Kd = nc.declare_dram_parameter("K", [HEADS_PER_CORE, D, S], f32, isOutput=False)
    # V host-relaid as [head, p, o, d] with s = o*128 + p so each SBUF
    # partition's load is one contiguous 8KB run.
    Vd = nc.declare_dram_parameter(
        "V", [HEADS_PER_CORE, 128, SB, D], f32, isOutput=False
    )
    # O stored partition-major [head, p, o, d] (s = o*128 + p) so each store
    # is one contiguous 8KB run per partition; host un-permutes after gather.
    Od = nc.declare_dram_parameter(
        "O", [HEADS_PER_CORE, 128, SB, D], f32, isOutput=True
    )

    # mm2 normalize/store groups of consecutive output blocks (3 fit a bank)
    GROUPS = [[0, 1, 2], [3, 4, 5], [6, 7, 8], [9, 10, 11], [12, 13, 14], [15]]

    with tile.TileContext(nc) as tc, ExitStack() as ctx:
        sb_pool = ctx.enter_context(tc.tile_pool(name="sb", bufs=2))
        o_pool = ctx.enter_context(tc.tile_pool(name="op", bufs=2))
        ps_pool = ctx.enter_context(tc.tile_pool(name="psp", bufs=2, space="PSUM"))
        po_pool = ctx.enter_context(tc.tile_pool(name="pop", bufs=2, space="PSUM"))
        const = in_pool = qk_pool = vp_pool = pt_pool = s_pool = sb_pool

        cf = const.tile([128, 3, 128], f32)
        eye = const.tile([128, 128], bf16)
        negtri = const.tile([128, 128], bf16)
        tri01 = const.tile([128, 128], bf16)
        warm_w = const.tile([128, 128], bf16)

        state = {}
        ps_tiles = {}

        def emit_loads(h):
            qtf = in_pool.tile([128, S], f32, tag="qtf")
            nc.sync.dma_start(qtf[:], Qd.ap()[h])
            ktf = in_pool.tile([128, S], f32, tag="ktf")
            nc.sync.dma_start(ktf[:], Kd.ap()[h])
            vn = in_pool.tile([128, SB, D], f32, tag="vn")
            nc.sync.dma_start(vn[:], Vd.ap()[h])
            state[h] = {"qtf": qtf, "ktf": ktf, "vn": vn}

        def emit_cast_q(h, half):
            st = state[h]
            if half == 0:
                st["qtb"] = qk_pool.tile([128, S], bf16, tag="qtb", name="qtb")
            sl = slice(half * (S // 2), (half + 1) * (S // 2))
            nc.vector.tensor_copy(st["qtb"][:, sl], st["qtf"][:, sl])

        def emit_cast_q_piece(h, c0, c1):
            st = state[h]
            if c0 == 0:
                st["qtb"] = qk_pool.tile([128, S], bf16, tag="qtb", name="qtb")
            nc.vector.tensor_copy(st["qtb"][:, c0:c1], st["qtf"][:, c0:c1])

        def emit_cast_k(h, half):
            st = state[h]
            if half == 0:
                st["ktb"] = qk_pool.tile([128, S], bf16, tag="ktb", name="ktb")
            sl = slice(half * (S // 2), (half + 1) * (S // 2))
            nc.vector.tensor_copy(st["ktb"][:, sl], st["ktf"][:, sl])

        def emit_cast_k_piece(h, c0, c1):
            st = state[h]
            if c0 == 0:
                st["ktb"] = qk_pool.tile([128, S], bf16, tag="ktb", name="ktb")
            nc.vector.tensor_copy(st["ktb"][:, c0:c1], st["ktf"][:, c0:c1])

        def emit_cast_v(h):
            st = state[h]
            vp = vp_pool.tile([128, SB, D + 4], bf16, tag="vp")
            nc.vector.tensor_copy(vp[:, :, 0:D], st["vn"][:])
            if h < 2:
                # the ones column survives slot reuse (casts only write 0:D)
                nc.vector.memset(vp[:, :, D : D + 1], 1.0)
            st["vp"] = vp

        # ---- mm2 job stream: one op per (block, contraction i) matmul, with
        # group-finalize ops (reciprocal + normalize + store) interleaved.
        # ready = global chunk step at which the needed pt slice is exp'd,
        # floored so a chain doesn't start long before its diagonal (keeps
        # the po3 psum slot hold short), plus a 2-step pipeline lag.
        def build_mm2_ops(h):
            ops = []
            for grp in GROUPS:
                for j, b in enumerate(grp):
                    rc_diag = _off(b) // CHUNK
                    lag = 2 if h + 1 < HEADS_PER_CORE else 0
                    for i in range(b + 1):
                        pos_rc = (_off(i) + 128 * (b - i)) // CHUNK
                        rdy = NCH * h + max(pos_rc, rc_diag - 3) + lag
                        ops.append((rdy, "mm", h, grp[0], len(grp), j, b, i))
                ops.append((ops[-1][0], "fin", h, grp[0], len(grp), 0, 0, 0))
            return ops

        mm2_ops = []
        for h in range(HEADS_PER_CORE):
            mm2_ops.extend(build_mm2_ops(h))
        mm2_cursor = [0]

        def emit_mm2(gstep, budget):
            cur = mm2_cursor[0]
            while cur < len(mm2_ops):
                rdy, kind, h, b0, glen, j, b, i = mm2_ops[cur]
                if rdy > gstep or (budget <= 0 and kind == "mm"):
                    break
                st = state[h]
                if kind == "mm":
                    if j == 0 and i == 0:
                        st["po3"] = po_pool.tile(
                            [128, 3, D + 4], f32, tag="po3", name="po3"
                        )
                    pos = _off(i) + 128 * (b - i)
                    nc.tensor.matmul(
                        st["po3"][:, j, 0 : D + 1],
                        lhsT=st["pt"][:, pos : pos + 128],
                        rhs=st["vp"][:, i, 0 : D + 1],
                        start=(i == 0),
                        stop=(i == b),
                        skip_group_check=True,
                    )
                    budget -= 1
                else:
                    po3 = st["po3"]
                    rec = s_pool.tile([128, 3], f32, tag="rec")
                    nc.vector.reciprocal(rec[:, 0:glen], po3[:, 0:glen, D])
                    if b0 == 0:
                        st["ob"] = o_pool.tile(
                            [128, SB, D], f32, tag="ob", name="ob"
                        )
                    nc.vector.tensor_tensor(
                        st["ob"][:, b0 : b0 + glen, :],
                        po3[:, 0:glen, 0:D],
                        rec[:, 0:glen, None].to_broadcast((128, glen, D)),
                        mybir.AluOpType.mult,
                    )
                    if b0 + glen == SB:
                        # one store per head: keeps the store queue shallow so
                        # the final store (which gates kernel end) fires fast
                        nc.gpsimd.dma_start(Od.ap()[h], st["ob"][:])
                cur += 1
            mm2_cursor[0] = cur

        def emit_mm1(h, c):
            st = state[h]
            if c == 0:
                st["pt"] = pt_pool.tile([128, FLAT], bf16, tag="pt", name="pt")
            qtb, ktb = st["qtb"], st["ktb"]
            s0 = CHUNK * c
            s1 = min(CHUNK * (c + 1), FLAT)
            ps = ps_pool.tile([128, CHUNK], f32, tag="ps")
            ps_tiles[NCH * h + c] = ps
            first_head_start = False
            # mm1: bank-aligned sub-matmuls over causal blocks in this chunk.
            # A block run starting at its diagonal gets a -1e30 seed first;
            # the first sub-matmul accumulates onto it (start=False). The
            # very first chunk skips its seed (no const dependency at start);
            # its diagonal is masked post-exp on the DVE instead.
            for i in range(SB):
                a = max(_off(i), s0)
                bnd = min(_off(i) + (S - 128 * i), s1)
                if a >= bnd:
                    continue
                seeded = a == _off(i) and not (h == 0 and c == 0)
                if seeded:
                    nc.tensor.matmul(
                        ps[:, a - s0 : a - s0 + 128],
                        lhsT=eye[:],
                        rhs=negtri[:],
                        start=True,
                        stop=False,
                        skip_group_check=True,
                    )
                f = a
                while f < bnd:
                    nxt = min(bnd, (f // 512 + 1) * 512)
                    q0 = 128 * i + (f - _off(i))
                    nc.tensor.matmul(
                        ps[:, f - s0 : nxt - s0],
                        lhsT=ktb[:, 128 * i : 128 * i + 128],
                        rhs=qtb[:, q0 : q0 + (nxt - f)],
                        start=not (seeded and f == a),
                        stop=True,
                        skip_group_check=True,
                    )
                    if first_head_start:
                        # stream exp piecewise so ACT starts ASAP at kernel start
                        nc.scalar.activation(
                            st["pt"][:, f:nxt],
                            ps[:, f - s0 : nxt - s0],
                            mybir.ActivationFunctionType.Exp,
                            scale=SCALE,
                        )
                    f = nxt

        def emit_exp(h, c):
            st = state[h]
            s0 = CHUNK * c
            s1 = min(CHUNK * (c + 1), FLAT)
            ps = ps_tiles.pop(NCH * h + c)
            nc.scalar.activation(
                st["pt"][:, s0:s1],
                ps[:, 0 : s1 - s0],
                mybir.ActivationFunctionType.Exp,
                scale=SCALE,
            )

        def emit_step(h, c):
            """Step g runs mm1 for chunk g+1 (one step ahead, so the mm2
            batch never sits between an exp and the mm1 it waits on), then
            exp for chunk g, then a paced mm2 batch."""
            gstep = NCH * h + c
            if c == 0 and h + 2 < HEADS_PER_CORE:
                emit_loads(h + 2)
            if h + 1 < HEADS_PER_CORE:
                if c == 4:
                    emit_cast_v(h + 1)
                elif c == 6:
                    emit_cast_q(h + 1, 0)
                elif c == 7:
                    emit_cast_q(h + 1, 1)
                elif c == 8:
                    emit_cast_k(h + 1, 0)
                elif c == 9:
                    emit_cast_k(h + 1, 1)

            if gstep > 0:
                emit_mm1(h, c)
                emit_exp(h, c)
            emit_mm2(gstep, MM2_BUDGET)

        # prologue. Ring order: Q0 first (longest pole: load -> cast -> mm1
        # -> exp), then K0's head block, consts, rest of K0, V0, head-1.
        # The PE warmup uses a memset tile so it only depends on GPSIMD.
        st0 = state.setdefault(0, {})
        qtf0 = in_pool.tile([128, S], f32, tag="qtf")
        ktf0 = in_pool.tile([128, S], f32, tag="ktf")
        nc.sync.dma_start(ktf0[:, 0:128], Kd.ap()[0][:, 0:128])
        nc.sync.dma_start(cf[:], Cd.ap().rearrange("c p d -> p c d"))
        nc.sync.dma_start(qtf0[:], Qd.ap()[0])
        nc.sync.dma_start(ktf0[:, 128:S], Kd.ap()[0][:, 128:S])
        vn0 = in_pool.tile([128, SB, D], f32, tag="vn")
        nc.sync.dma_start(vn0[:], Vd.ap()[0])
        st0.update({"qtf": qtf0, "ktf": ktf0, "vn": vn0})
        emit_loads(1)

        nc.gpsimd.memset(warm_w[:], 0.5)
        # load the exp table on ACT during the input loads (tiny dummy exp)
        warm_act = s_pool.tile([128, 1], f32, tag="wa")
        nc.scalar.activation(
            warm_act[:], warm_w[:, 0:1], mybir.ActivationFunctionType.Exp,
            scale=SCALE,
        )
        # PE warmup: start the p-state ramp before the first real matmul
        wps = ps_pool.tile([128, CHUNK], f32, tag="ps")
        for _ in range(N_WARM):
            nc.tensor.matmul(
                wps[:, 0:128], lhsT=warm_w[:], rhs=warm_w[:], start=True,
                stop=True, skip_group_check=True,
            )

        emit_cast_q_piece(0, 0, CHUNK)
        emit_cast_k_piece(0, 0, 128)
        nc.vector.tensor_copy(eye[:], cf[:, 0, :])
        nc.vector.tensor_copy(negtri[:], cf[:, 1, :])
        nc.vector.tensor_copy(tri01[:], cf[:, 2, :])
        emit_cast_q_piece(0, CHUNK, S)
        emit_cast_k_piece(0, 128, 1024)
        emit_cast_k_piece(0, 1024, S)
        emit_cast_v(0)
        emit_mm1(0, 0)
        emit_exp(0, 0)
        # chunk (0,0) carried no seed: mask its diagonal block post-exp
        nc.vector.tensor_tensor(
            st0["pt"][:, 0:128], st0["pt"][:, 0:128], tri01[:],
            mybir.AluOpType.mult,
        )

        for h in range(HEADS_PER_CORE):
            for c in range(NCH):
                emit_step(h, c)
        # drain the mm2 tail
        emit_mm2(10**9, 10**9)

    nc.compile()
    return nc


def _get_nc():
    if "nc" not in _CACHE:
        _CACHE["nc"] = _build()
    return _CACHE["nc"]


def _consts():
    eye = np.eye(128, dtype=np.float32)
    lower = np.arange(128)[:, None] > np.arange(128)[None, :]
    negtri = np.where(lower, -1e30, 0.0).astype(np.float32)
    tri01 = np.where(lower, 0.0, 1.0).astype(np.float32)
    return np.stack([eye, negtri, tri01])


def _in_maps(Q, K, V):
    """Host-side shard + layout prep: Q,K -> [head, d, s], V -> [head, s, d]."""
    Qf = np.asarray(Q, dtype=np.float32).reshape(B * H, S, D)
    Kf = np.asarray(K, dtype=np.float32).reshape(B * H, S, D)
    Vf = np.ascontiguousarray(
        np.asarray(V, dtype=np.float32)
        .reshape(B * H, SB, 128, D)
        .transpose(0, 2, 1, 3)
    )
    Qt = np.ascontiguousarray(Qf.transpose(0, 2, 1))
    Kt = np.ascontiguousarray(Kf.transpose(0, 2, 1))
    C = _consts()
    maps = []
    for c in range(N_CORES):
        sl = slice(c * HEADS_PER_CORE, (c + 1) * HEADS_PER_CORE)
        maps.append({"C": C, "Q": Qt[sl], "K": Kt[sl], "V": Vf[sl]})
    return maps


def _gather(res):
    out = np.concatenate(
        [res.results[c]["O"] for c in range(N_CORES)], axis=0
    )
    # [bh, p, o, d] -> [bh, s, d] with s = o*128 + p
    out = out.reshape(B * H, 128, SB, D).transpose(0, 2, 1, 3)
    return np.ascontiguousarray(out).reshape(B, H, S, D).astype(np.float32)


def kernel(Q: np.ndarray, K: np.ndarray, V: np.ndarray) -> np.ndarray:
    from concourse.bass_utils import run_bass_kernel_spmd

    nc = _get_nc()
    res = run_bass_kernel_spmd(nc, _in_maps(Q, K, V), core_ids=list(range(N_CORES)))
    return _gather(res)


# revision 65
# speedup vs baseline: 1.2042x; 1.0282x over previous
"""Causal multi-head attention for Trainium2, sharded over 8 NeuronCores.

Problem: Q,K,V [2, 16, 2048, 128] fp32 -> O [2, 16, 2048, 128] fp32
  scores = (Q @ K^T) / sqrt(128), causal mask, softmax, @ V.

Sharding: the 32 (batch, head) slices are data-parallel; each of the 8
cores computes 4 heads independently (no collectives). Host-side layout
prep per head: Q,K are pre-transposed to [d, s] (the PE contraction dim d
lands on partitions) and pre-cast to bf16 (the same rounding the on-chip
matmul feed would apply); V is pre-cast, gets a ones column appended (the
softmax denominator rides along mm2), and is relaid partition-major.

Per-head dataflow on one core (S=2048, D=128, bf16 matmuls, fp32 psum):
  mm1 computes scores^T [k, q] over the causal region only, packed into a
  flat 17408-col buffer (block i occupies cols off(i)..off(i)+2048-128*i)
  as 512-col psum-bank-aligned sub-matmuls; each diagonal block's psum is
  pre-seeded with -1e30 on the strict lower triangle so exp emits exact
  zeros there (the very first chunk instead masks post-exp on the DVE to
  cut the startup dependency chain). ACT exps 1536-col chunks (12/head,
  scale folded, fp32 psum in / bf16 out; no max-subtraction: scores are
  O(+-8)). mm2 per 128-row output block b accumulates pt-stationary
  matmuls over [V | 1]; reciprocal+normalize batched 3 blocks per psum
  bank into a per-head output tile; one contiguous store per head.

Queues: Sync = loads, Scalar = exp only, GPSIMD = stores (SWDGE),
DVE = normalize, PE = mm1 + seeds + mm2 (+ warmup during the loads).
A global budget-paced mm2 cursor runs a few chunks behind exp; loads
prefetch two heads ahead; exp runs gapless in steady state (~71us of a
~96us kernel; ACT is the binding engine, PE is ~93% busy).
"""

import math
from contextlib import ExitStack

import numpy as np

N_CORES = 8
B, H, S, D = 2, 16, 2048, 128
HEADS_PER_CORE = (B * H) // N_CORES  # 4
SB = S // 128  # 16 k-blocks per head
SCALE = 1.0 / math.sqrt(128.0)
CHUNK = 1536
FLAT = sum(S - 128 * i for i in range(SB))  # 17408
NCH = (FLAT + CHUNK - 1) // CHUNK  # 12 exp chunks per head
N_WARM = 60  # PE warmup matmuls (post-program-load, pre-first-mm1)
MM2_BUDGET = 13  # mm2 matmuls emitted per chunk step

_CACHE = {}


def _off(i):
    # flat column offset of k-block i's causal q-range (width S - 128*i)
    return 2048 * i - 64 * i * (i - 1)


def _build():
    import concourse.bass as bass  # noqa: F401
    import concourse.tile as tile
    from concourse import bacc, mybir

    f32 = mybir.dt.float32
    bf16 = mybir.dt.bfloat16

    nc = bacc.Bacc("TRN2", num_devices=N_CORES)
    # C: host consts: [0]=identity, [1]=-1e30 strict lower tri, [2]=0/1 keep
    Cd = nc.declare_dram_parameter("C", [3, 128, 128], f32, isOutput=False)
    Qd = nc.declare_dram_parameter("Q", [HEADS_PER_CORE, D, S], bf16, isOutput=False)
    Kd = nc.declare_dram_parameter("K", [HEADS_PER_CORE, D, S], bf16, isOutput=False)
    # V relaid [head, p, o, d+4] with s = o*128 + p and a ones column at
    # d=D, so each partition's load is one contiguous run.
    Vd = nc.declare_dram_parameter(
        "V", [HEADS_PER_CORE, 128, SB, D + 4], bf16, isOutput=False
    )
    # O partition-major [head, p, o, d] (s = o*128 + p): contiguous stores;
    # the host un-permutes after gather.
    Od = nc.declare_dram_parameter(
        "O", [HEADS_PER_CORE, 128, SB, D], f32, isOutput=True
    )

    # mm2 normalize/store groups of consecutive output blocks (3 fit a bank)
    GROUPS = [[0, 1, 2], [3, 4, 5], [6, 7, 8], [9, 10, 11], [12, 13, 14], [15]]

    with tile.TileContext(nc) as tc, ExitStack() as ctx:
        sb_pool = ctx.enter_context(tc.tile_pool(name="sb", bufs=2))
        o_pool = ctx.enter_context(tc.tile_pool(name="op", bufs=2))
        ps_pool = ctx.enter_context(tc.tile_pool(name="psp", bufs=2, space="PSUM"))
        po_pool = ctx.enter_context(tc.tile_pool(name="pop", bufs=2, space="PSUM"))
        const = in_pool = pt_pool = s_pool = sb_pool

        cf = const.tile([128, 3, 128], f32)
        eye = const.tile([128, 128], bf16)
        negtri = const.tile([128, 128], bf16)
        tri01 = const.tile([128, 128], bf16)
        warm_w = const.tile([128, 128], bf16)

        state = {}
        ps_tiles = {}

        def emit_loads(h):
            qtb = in_pool.tile([128, S], bf16, tag="qtb")
            nc.sync.dma_start(qtb[:], Qd.ap()[h])
            ktb = in_pool.tile([128, S], bf16, tag="ktb")
            nc.sync.dma_start(ktb[:], Kd.ap()[h])
            vp = in_pool.tile([128, SB, D + 4], bf16, tag="vp")
            nc.sync.dma_start(vp[:], Vd.ap()[h])
            state[h] = {"qtb": qtb, "ktb": ktb, "vp": vp}

        # ---- mm2 job stream: one op per (block, contraction i) matmul, with
        # group-finalize ops (reciprocal + normalize + store) interleaved.
        # ready = global chunk step at which the needed pt slice is exp'd,
        # floored so a chain doesn't start long before its diagonal (keeps
        # the po3 psum slot hold short), plus a 2-step pipeline lag (0 for
        # the last head so its tail drains during the final exps).
        def build_mm2_ops(h):
            ops = []
            for grp in GROUPS:
                for j, b in enumerate(grp):
                    rc_diag = _off(b) // CHUNK
                    lag = 2 if h + 1 < HEADS_PER_CORE else 0
                    for i in range(b + 1):
                        pos_rc = (_off(i) + 128 * (b - i)) // CHUNK
                        rdy = NCH * h + max(pos_rc, rc_diag - 3) + lag
                        ops.append((rdy, "mm", h, grp[0], len(grp), j, b, i))
                ops.append((ops[-1][0], "fin", h, grp[0], len(grp), 0, 0, 0))
            return ops

        mm2_ops = []
        for h in range(HEADS_PER_CORE):
            mm2_ops.extend(build_mm2_ops(h))
        mm2_cursor = [0]

        def emit_mm2(gstep, budget):
            cur = mm2_cursor[0]
            while cur < len(mm2_ops):
                rdy, kind, h, b0, glen, j, b, i = mm2_ops[cur]
                if rdy > gstep or (budget <= 0 and kind == "mm"):
                    break
                st = state[h]
                if kind == "mm":
                    if j == 0 and i == 0:
                        st["po3"] = po_pool.tile(
                            [128, 3, D + 4], f32, tag="po3", name="po3"
                        )
                    pos = _off(i) + 128 * (b - i)
                    nc.tensor.matmul(
                        st["po3"][:, j, 0 : D + 1],
                        lhsT=st["pt"][:, pos : pos + 128],
                        rhs=st["vp"][:, i, 0 : D + 1],
                        start=(i == 0),
                        stop=(i == b),
                        skip_group_check=True,
                    )
                    budget -= 1
                else:
                    po3 = st["po3"]
                    rec = s_pool.tile([128, 3], f32, tag="rec")
                    nc.vector.reciprocal(rec[:, 0:glen], po3[:, 0:glen, D])
                    if b0 == 0:
                        st["ob"] = o_pool.tile(
                            [128, SB, D], f32, tag="ob", name="ob"
                        )
                    nc.vector.tensor_tensor(
                        st["ob"][:, b0 : b0 + glen, :],
                        po3[:, 0:glen, 0:D],
                        rec[:, 0:glen, None].to_broadcast((128, glen, D)),
                        mybir.AluOpType.mult,
                    )
                    if b0 + glen == SB:
                        # one store per head: keeps the store queue shallow so
                        # the final store (which gates kernel end) fires fast
                        nc.gpsimd.dma_start(Od.ap()[h], st["ob"][:])
                cur += 1
            mm2_cursor[0] = cur

        def emit_mm1(h, c):
            st = state[h]
            if c == 0:
                st["pt"] = pt_pool.tile([128, FLAT], bf16, tag="pt", name="pt")
            qtb, ktb = st["qtb"], st["ktb"]
            s0 = CHUNK * c
            s1 = min(CHUNK * (c + 1), FLAT)
            ps = ps_pool.tile([128, CHUNK], f32, tag="ps")
            ps_tiles[NCH * h + c] = ps
            # mm1: bank-aligned sub-matmuls over causal blocks in this chunk.
            # A block run starting at its diagonal gets a -1e30 seed first;
            # the first sub-matmul accumulates onto it (start=False). The
            # very first chunk skips its seed (no const dependency at start);
            # its diagonal is masked post-exp on the DVE instead.
            for i in range(SB):
                a = max(_off(i), s0)
                bnd = min(_off(i) + (S - 128 * i), s1)
                if a >= bnd:
                    continue
                seeded = a == _off(i) and not (h == 0 and c == 0)
                if seeded:
                    nc.tensor.matmul(
                        ps[:, a - s0 : a - s0 + 128],
                        lhsT=eye[:],
                        rhs=negtri[:],
                        start=True,
                        stop=False,
                        skip_group_check=True,
                    )
                f = a
                while f < bnd:
                    nxt = min(bnd, (f // 512 + 1) * 512)
                    q0 = 128 * i + (f - _off(i))
                    nc.tensor.matmul(
                        ps[:, f - s0 : nxt - s0],
                        lhsT=ktb[:, 128 * i : 128 * i + 128],
                        rhs=qtb[:, q0 : q0 + (nxt - f)],
                        start=not (seeded and f == a),
                        stop=True,
                        skip_group_check=True,
                    )
                    f = nxt

        def emit_exp(h, c):
            st = state[h]
            s0 = CHUNK * c
            s1 = min(CHUNK * (c + 1), FLAT)
            ps = ps_tiles.pop(NCH * h + c)
            nc.scalar.activation(
                st["pt"][:, s0:s1],
                ps[:, 0 : s1 - s0],
                mybir.ActivationFunctionType.Exp,
                scale=SCALE,
            )

        def emit_step(h, c):
            gstep = NCH * h + c
            if c == 0 and h + 2 < HEADS_PER_CORE:
                emit_loads(h + 2)
            if gstep > 0:
                emit_mm1(h, c)
                emit_exp(h, c)
            emit_mm2(gstep, MM2_BUDGET)

        # prologue, ordered by first use on the sync ring: K0's head block,
        # consts, Q0, rest of K0, V0, head-1. The PE warmup uses a memset
        # tile so it depends only on GPSIMD.
        st0 = state.setdefault(0, {})
        ktb0 = in_pool.tile([128, S], bf16, tag="ktb")
        nc.sync.dma_start(ktb0[:, 0:128], Kd.ap()[0][:, 0:128])
        nc.sync.dma_start(cf[:], Cd.ap().rearrange("c p d -> p c d"))
        qtb0 = in_pool.tile([128, S], bf16, tag="qtb")
        nc.sync.dma_start(qtb0[:], Qd.ap()[0])
        nc.sync.dma_start(ktb0[:, 128:S], Kd.ap()[0][:, 128:S])
        vp0 = in_pool.tile([128, SB, D + 4], bf16, tag="vp")
        nc.sync.dma_start(vp0[:], Vd.ap()[0])
        st0.update({"qtb": qtb0, "ktb": ktb0, "vp": vp0})
        emit_loads(1)

        nc.gpsimd.memset(warm_w[:], 0.5)
        # load the exp table on ACT during the input loads (tiny dummy exp)
        warm_act = s_pool.tile([128, 1], f32, tag="wa")
        nc.scalar.activation(
            warm_act[:], warm_w[:, 0:1], mybir.ActivationFunctionType.Exp,
            scale=SCALE,
        )
        # PE warmup: start the p-state ramp before the first real matmul
        wps = ps_pool.tile([128, CHUNK], f32, tag="ps")
        for _ in range(N_WARM):
            nc.tensor.matmul(
                wps[:, 0:128], lhsT=warm_w[:], rhs=warm_w[:], start=True,
                stop=True, skip_group_check=True,
            )

        nc.vector.tensor_copy(eye[:], cf[:, 0, :])
        nc.vector.tensor_copy(negtri[:], cf[:, 1, :])
        nc.vector.tensor_copy(tri01[:], cf[:, 2, :])
        emit_mm1(0, 0)
        emit_exp(0, 0)
        # chunk (0,0) carried no seed: mask its diagonal block post-exp
        nc.vector.tensor_tensor(
            st0["pt"][:, 0:128], st0["pt"][:, 0:128], tri01[:],
            mybir.AluOpType.mult,
        )

        for h in range(HEADS_PER_CORE):
            for c in range(NCH):
                emit_step(h, c)
        # drain the mm2 tail
        emit_mm2(10**9, 10**9)

    nc.compile()
    return nc


def _get_nc():
    if "nc" not in _CACHE:
        _CACHE["nc"] = _build()
    return _CACHE["nc"]


def _consts():
    eye = np.eye(128, dtype=np.float32)
    lower = np.arange(128)[:, None] > np.arange(128)[None, :]
    negtri = np.where(lower, -1e30, 0.0).astype(np.float32)
    tri01 = np.where(lower, 0.0, 1.0).astype(np.float32)
    return np.stack([eye, negtri, tri01])


def _in_maps(Q, K, V):
    """Host-side shard + layout prep: Q,K -> bf16 [head, d, s]; V -> bf16
    [head, p, o, d+4] with a ones column at d=D; consts."""
    import ml_dtypes

    bf16 = ml_dtypes.bfloat16
    Qf = np.asarray(Q, dtype=np.float32).reshape(B * H, S, D)
    Kf = np.asarray(K, dtype=np.float32).reshape(B * H, S, D)
    Vf = np.asarray(V, dtype=np.float32).reshape(B * H, S, D)
    Qt = np.ascontiguousarray(Qf.transpose(0, 2, 1)).astype(bf16)
    Kt = np.ascontiguousarray(Kf.transpose(0, 2, 1)).astype(bf16)
    Vx = np.zeros((B * H, S, D + 4), dtype=bf16)
    Vx[:, :, 0:D] = Vf.astype(bf16)
    Vx[:, :, D] = bf16(1.0)
    Vx = np.ascontiguousarray(
        Vx.reshape(B * H, SB, 128, D + 4).transpose(0, 2, 1, 3)
    )
    C = _consts()
    maps = []
    for c in range(N_CORES):
        sl = slice(c * HEADS_PER_CORE, (c + 1) * HEADS_PER_CORE)
        maps.append({"C": C, "Q": Qt[sl], "K": Kt[sl], "V": Vx[sl]})
    return maps


def _gather(res):
    out = np.concatenate(
        [res.results[c]["O"] for c in range(N_CORES)], axis=0
    )
    # [bh, p, o, d] -> [bh, s, d] with s = o*128 + p
    out = out.reshape(B * H, 128, SB, D).transpose(0, 2, 1, 3)
    return np.ascontiguousarray(out).reshape(B, H, S, D).astype(np.float32)


def kernel(Q: np.ndarray, K: np.ndarray, V: np.ndarray) -> np.ndarray:
    from concourse.bass_utils import run_bass_kernel_spmd

    nc = _get_nc()
    res = run_bass_kernel_spmd(nc, _in_maps(Q, K, V), core_ids=list(range(N_CORES)))
    return _gather(res)
